# revision 13
# baseline (speedup 1.0000x reference)
"""Trainium2 Bass kernel for nn_LoraLinear (embedding_lookup, 8 cores).

Computation (per batch row b):
    out[b] = x[b] @ W_base.T + b_base
             + S * ( (B_user[u_b] + B_item[i_b] + W_common) @ (x[b] @ A.T) )
with S = 2.0, shapes: x [4096,1024], tables [10000,1024,16], A [16,1024],
W_common [1024,16], out [4096,1024].

Strategy: pure data-parallel over the batch (512 rows/core); B tables are
replicated in each core's HBM (fp8) and rows are fetched with indirect
DMA gathers (one 2 MiB gather per 128 batch rows per table; macro-row =
the full [1024,16] fp8 matrix for one id). The rank-16 per-row "matvec"
runs on the TensorEngine as diagonal-lhsT matmuls: 128 batch rows per
matmul (partition j <-> batch row j), one r-value per k-slot, r-sum
completed by accumulating r-chunk matmuls into PSUM; with DR=True two
r-values ride the fp8 DoubleRow k-subtile dimension (2x PE throughput).
The base matmul (bf16), bias (K=1 matmul) and common-part matmul
accumulate into the same 8 PSUM banks. No collectives.

Numerics: base/bias/common in bf16 (fp32 accumulate); tables in fp8
(e4m3 when DR, e3m4 x64 otherwise); lora lhsT coefficients in fp8-e4m3
(DR) or bf16. Measured end-to-end max rel err ~1.4e-2 (DR) / ~4e-3.

Host-side prep (not on the accelerator): layout transposes, fp8/bf16
casts of the weights/tables, and index copies only.
"""
import numpy as np
import ml_dtypes

import concourse.bass as bass
import concourse.bacc as bacc
import concourse.tile as tile
from concourse import mybir
from concourse.bass_utils import run_bass_kernel_spmd

# problem shapes (hardcoded per contract)
IN_F = 1024
OUT_F = 1024
R = 16
NUM_USERS = 10000
NUM_ITEMS = 10000
BATCH = 4096
SCALING = 2.0
N_CORES = 8

B_SH = BATCH // N_CORES          # 512 rows per core
RG = 128                         # batch rows per matmul group (= partitions)
NG = B_SH // RG                  # 4 groups per core
NKC = IN_F // 128                # 8 contraction chunks for the base matmul
NH = OUT_F // 512                # 2 output halves (PSUM bank free-dim limit)
NBG = B_SH // 128                # 4 PSUM row-blocks (== NG)

DR = True                        # fp8-e4m3 DoubleRow for the lora matmuls
DR_SUB = 2 if DR else 1          # k-subtiles per matmul (DoubleRow pairs)
C_SUB = R // DR_SUB              # r-chunks accumulated via separate matmuls
TAB_SCALE = 1.0 if DR else 64.0  # fp8 range centering (1/scale folded in ltab)

F32 = mybir.dt.float32
BF16 = mybir.dt.bfloat16
F8E4 = mybir.dt.float8e4         # e4m3
F8E3 = mybir.dt.float8e3         # e3m4
I32 = mybir.dt.int32
TAB_DT = F8E4 if DR else F8E3
BT_DT = F8E4 if DR else BF16
MACRO = R * OUT_F                # fp8 elements per gathered macro-row

_CACHE = {}


def _build(reps=1):
    nc = bacc.Bacc("TRN2", target_bir_lowering=False, debug=False,
                   num_devices=N_CORES)
    xt = nc.dram_tensor("xt", [IN_F, B_SH], BF16, kind="ExternalInput")
    wt = nc.dram_tensor("wt", [IN_F, OUT_F], BF16, kind="ExternalInput")
    a2w = nc.dram_tensor("a2w", [128, NKC * R], BF16, kind="ExternalInput")
    wct = nc.dram_tensor("wct", [R, OUT_F], BF16, kind="ExternalInput")
    biasb = nc.dram_tensor("biasb", [1, OUT_F], BF16, kind="ExternalInput")
    ones1 = nc.dram_tensor("ones1", [1, 128], BF16, kind="ExternalInput")
    ltab = nc.dram_tensor("ltab", [R, R * 128], BF16, kind="ExternalInput")
    masks = nc.dram_tensor("masks", [128, NG * RG], BF16, kind="ExternalInput")
    but = nc.dram_tensor("but", [NUM_USERS, MACRO], TAB_DT,
                         kind="ExternalInput")
    bit = nc.dram_tensor("bit", [NUM_ITEMS, MACRO], TAB_DT,
                         kind="ExternalInput")
    uidx = nc.dram_tensor("uidx", [128, NG], I32, kind="ExternalInput")
    iidx = nc.dram_tensor("iidx", [128, NG], I32, kind="ExternalInput")
    # y layout: [p, (bg, h, o)] -> row 128*bg+p, col 512*h+o of the [512,
    # 1024] shard. One contiguous 16 KiB row per partition => the per-body
    # output store is a single DMA with 16 KiB descriptors (128x 2 KiB
    # descriptors per bank would run at ~1/5 the engine rate).
    y = nc.dram_tensor("y", [128, NBG * OUT_F], F32, kind="ExternalOutput")

    perf_mode = mybir.MatmulPerfMode.DoubleRow if DR else None

    with tile.TileContext(nc) as tc:
        with (
            tc.tile_pool(name="const", bufs=1) as cp,
            tc.tile_pool(name="work", bufs=2) as wp,
            tc.tile_pool(name="gath", bufs=6) as gp,
            tc.tile_pool(name="ps", bufs=8, space="PSUM") as psp,
            tc.tile_pool(name="outp", bufs=2) as op,
        ):
            # ---- constant / weight loads (once) ----
            xt_t = []
            for k in range(NKC):
                t = cp.tile([128, B_SH], BF16, tag=f"xt{k}")
                nc.sync.dma_start(t[:], xt.ap()[128 * k:128 * (k + 1), :])
                xt_t.append(t)
            wt_t = []
            for k in range(NKC):
                t = cp.tile([128, OUT_F], BF16, tag=f"wt{k}")
                nc.sync.dma_start(t[:], wt.ap()[128 * k:128 * (k + 1), :])
                wt_t.append(t)
            a2w_t = cp.tile([128, NKC * R], BF16, tag="a2w")
            nc.sync.dma_start(a2w_t[:], a2w.ap())
            wct_t = cp.tile([R, OUT_F], BF16, tag="wct")
            nc.sync.dma_start(wct_t[:], wct.ap())
            bias_t = cp.tile([1, OUT_F], BF16, tag="bias")
            nc.sync.dma_start(bias_t[:], biasb.ap())
            ones_t = cp.tile([1, 128], BF16, tag="ones")
            nc.sync.dma_start(ones_t[:], ones1.ap())
            ltab_t = cp.tile([R, R * 128], BF16, tag="ltab")
            nc.sync.dma_start(ltab_t[:], ltab.ap())
            mask_t = cp.tile([128, NG, RG], BF16, tag="mask")
            nc.sync.dma_start(mask_t[:], masks.ap())
            uidx_t = cp.tile([128, NG], I32, tag="uidx")
            nc.sync.dma_start(uidx_t[:], uidx.ap())
            iidx_t = cp.tile([128, NG], I32, tag="iidx")
            nc.sync.dma_start(iidx_t[:], iidx.ap())

            def body():
                ot_all = op.tile([128, NBG, NH, 512], F32, tag="ot")

                # ---- a2T = (2A) @ x_shard.T  -> [16, 512] f32 -> bf16 ----
                a2t_ps = psp.tile([128, 512], F32, tag="ps", space="PSUM")
                for k in range(NKC):
                    nc.tensor.matmul(
                        a2t_ps[:R, :], lhsT=a2w_t[:, R * k:R * (k + 1)],
                        rhs=xt_t[k][:],
                        start=(k == 0), stop=(k == NKC - 1),
                        skip_group_check=True)
                a2t_sb = wp.tile([R, B_SH], BF16, tag="a2t")
                nc.vector.tensor_copy(a2t_sb[:], a2t_ps[:R, :])

                # ---- bt: diagonal lhsT coefficients (batch row j <-> col j)
                # rep matmul broadcasts a2[ci] across partitions into PSUM;
                # the mask-mult reads it straight from PSUM.
                bt_all = []
                for c in range(C_SUB):
                    bt = wp.tile([128, NG, DR_SUB, RG], BT_DT, tag=f"bt{c}")
                    for i in range(DR_SUB):
                        ci = DR_SUB * c + i
                        rps = psp.tile([128, NG, RG], F32, tag="ps",
                                       space="PSUM")
                        nc.tensor.matmul(
                            rps[:].opt(), lhsT=ltab_t[:, 128 * ci:128 * (ci + 1)],
                            rhs=a2t_sb[:],
                            start=True, stop=True, skip_group_check=True)
                        nc.vector.tensor_tensor(
                            out=bt[:, :, i, :], in0=mask_t[:],
                            in1=rps[:],
                            op=mybir.AluOpType.mult)
                    bt_all.append(bt)

                # ---- per group: init PSUM bank (bias+base+common), gather
                # both table rows, run the diagonal lora matmuls, store. The
                # bank init is interleaved with the g loop so the PE reaches
                # the first lora matmul quickly and gather buffers recycle
                # without stalling the DMA engines.
                for g in range(NG):
                    ps_h = {}
                    for h in range(NH):
                        ps = psp.tile([128, 512], F32, tag="ps", space="PSUM")
                        ps_h[h] = ps
                        nc.tensor.matmul(  # bias broadcast (K=1)
                            ps[:], lhsT=ones_t[:],
                            rhs=bias_t[:, 512 * h:512 * h + 512],
                            start=True, stop=False, skip_group_check=True)
                        for k in range(NKC):  # base: x @ W_base.T (bf16)
                            nc.tensor.matmul(
                                ps[:], lhsT=xt_t[k][:, 128 * g:128 * (g + 1)],
                                rhs=wt_t[k][:, 512 * h:512 * h + 512],
                                start=False, stop=False, skip_group_check=True)
                        nc.tensor.matmul(  # common: a2 @ W_common.T
                            ps[:], lhsT=a2t_sb[:, 128 * g:128 * (g + 1)],
                            rhs=wct_t[:, 512 * h:512 * h + 512],
                            start=False, stop=False, skip_group_check=True)
                    n_left = {h: 2 * C_SUB for h in range(NH)}
                    for tab_ap, idx_t in ((but.ap(), uidx_t), (bit.ap(), iidx_t)):
                        gt = gp.tile([128, C_SUB, DR_SUB, OUT_F], TAB_DT,
                                     tag="gt")
                        nc.gpsimd.indirect_dma_start(
                            out=gt[:].opt(), out_offset=None, in_=tab_ap,
                            in_offset=bass.IndirectOffsetOnAxis(
                                ap=idx_t[:, g:g + 1], axis=0))
                        for c in range(C_SUB):
                            for h in range(NH):
                                n_left[h] -= 1
                                nc.tensor.matmul(
                                    ps_h[h][:],
                                    lhsT=bt_all[c][:, g, :, :],
                                    rhs=gt[:, c, :, 512 * h:512 * h + 512],
                                    start=False, stop=(n_left[h] == 0),
                                    perf_mode=perf_mode,
                                    skip_group_check=True)
                    for h in range(NH):
                        nc.scalar.copy(
                            ot_all[:, g, h, :], ps_h[h][:])
                nc.sync.dma_start(y.ap(), ot_all[:].opt())

            for _ in range(reps):
                body()
    nc.compile()
    return nc


def _prep_host(x, user_indices, item_indices, W_base, b_base, A, B_user,
               B_item, W_common):
    """Host-side layout prep. Returns (shared dict, per-core list of dicts)."""
    bf16 = ml_dtypes.bfloat16
    tab_np = mybir.dt.np(TAB_DT)
    x = np.asarray(x, np.float32)
    W_base = np.asarray(W_base, np.float32)
    b_base = np.asarray(b_base, np.float32)
    A = np.asarray(A, np.float32)
    W_common = np.asarray(W_common, np.float32)
    user_indices = np.asarray(user_indices, np.int32)
    item_indices = np.asarray(item_indices, np.int32)

    wt = np.ascontiguousarray(W_base.T).astype(bf16)          # [in, out]
    a2t = np.ascontiguousarray((SCALING * A).T)               # [in, R]
    # a2w[p, R*k + r] = a2t[128k + p, r]
    a2w = np.ascontiguousarray(
        a2t.reshape(NKC, 128, R).transpose(1, 0, 2).reshape(128, NKC * R)
    ).astype(bf16)
    wct = np.ascontiguousarray(W_common.T).astype(bf16)       # [R, out]
    biasb = b_base.reshape(1, OUT_F).astype(bf16)
    ones1 = np.ones((1, 128), bf16)
    # ltab[r, 128*ci + p] = 1/TAB_SCALE iff r == ci  (broadcast a2[ci])
    ltab = np.zeros((R, R * 128), np.float32)
    for ci in range(R):
        ltab[ci, 128 * ci:128 * (ci + 1)] = 1.0 / TAB_SCALE
    ltab = ltab.astype(bf16)
    # masks[p, g, j] = 1 if p == j  (diagonal; same for every group)
    masks = np.zeros((128, NG, RG), np.float32)
    p = np.arange(128)
    masks[p, :, p] = 1.0
    masks = masks.reshape(128, NG * RG).astype(bf16)

    def mk_table(B, n):
        # [U, O, R] -> [U, R, O] with r-order (c, i): r = DR_SUB*c + i
        Bq = (np.asarray(B, np.float32) * TAB_SCALE).astype(tab_np)
        return np.ascontiguousarray(
            Bq.transpose(0, 2, 1)).reshape(n, MACRO)

    but = mk_table(B_user, NUM_USERS)
    bit = mk_table(B_item, NUM_ITEMS)

    shared = dict(wt=wt, a2w=a2w, wct=wct, biasb=np.asarray(biasb),
                  ones1=np.asarray(ones1), ltab=ltab, masks=masks,
                  but=but, bit=bit)
    per_core = []
    for cc in range(N_CORES):
        sl = slice(B_SH * cc, B_SH * (cc + 1))
        xt_c = np.ascontiguousarray(x[sl].T).astype(bf16)     # [in, 512]
        per_core.append(dict(
            xt=xt_c,
            uidx=np.ascontiguousarray(
                user_indices[sl].reshape(NG, RG).T),
            iidx=np.ascontiguousarray(
                item_indices[sl].reshape(NG, RG).T)))
    return shared, per_core


def kernel(**inputs) -> np.ndarray:
    if "nc" not in _CACHE:
        _CACHE["nc"] = _build()
    nc = _CACHE["nc"]
    shared, per_core = _prep_host(**inputs)
    in_maps = [{**shared, **pc} for pc in per_core]
    res = run_bass_kernel_spmd(nc, in_maps, core_ids=list(range(N_CORES)))
    shards = [_unshard_y(res.results[c]["y"]) for c in range(N_CORES)]
    return np.concatenate(shards, axis=0).astype(np.float32)


def _unshard_y(y_dev):
    return (y_dev.reshape(128, NBG, NH, 512)
            .transpose(1, 0, 2, 3).reshape(B_SH, OUT_F))


# revision 14
# speedup vs baseline: 1.0019x; 1.0019x over previous
"""Trainium2 Bass kernel for nn_LoraLinear (embedding_lookup, 8 cores).

Computation (per batch row b):
    out[b] = x[b] @ W_base.T + b_base
             + S * ( (B_user[u_b] + B_item[i_b] + W_common) @ (x[b] @ A.T) )
with S = 2.0, shapes: x [4096,1024], tables [10000,1024,16], A [16,1024],
W_common [1024,16], out [4096,1024].

Strategy: pure data-parallel over the batch (512 rows/core); B tables are
replicated in each core's HBM (fp8) and rows are fetched with indirect
DMA gathers (one 2 MiB gather per 128 batch rows per table; macro-row =
the full [1024,16] fp8 matrix for one id). The rank-16 per-row "matvec"
runs on the TensorEngine as diagonal-lhsT matmuls: 128 batch rows per
matmul (partition j <-> batch row j), one r-value per k-slot, r-sum
completed by accumulating r-chunk matmuls into PSUM; with DR=True two
r-values ride the fp8 DoubleRow k-subtile dimension (2x PE throughput).
The base matmul (bf16), bias (K=1 matmul) and common-part matmul
accumulate into the same 8 PSUM banks. No collectives.

Numerics: base/bias/common in bf16 (fp32 accumulate); tables in fp8
(e4m3 when DR, e3m4 x64 otherwise); lora lhsT coefficients in fp8-e4m3
(DR) or bf16. Measured end-to-end max rel err ~1.4e-2 (DR) / ~4e-3.

Host-side prep (not on the accelerator): layout transposes, fp8/bf16
casts of the weights/tables, and index copies only.
"""
import numpy as np
import ml_dtypes

import concourse.bass as bass
import concourse.bacc as bacc
import concourse.tile as tile
from concourse import mybir
from concourse.bass_utils import run_bass_kernel_spmd

# problem shapes (hardcoded per contract)
IN_F = 1024
OUT_F = 1024
R = 16
NUM_USERS = 10000
NUM_ITEMS = 10000
BATCH = 4096
SCALING = 2.0
N_CORES = 8

B_SH = BATCH // N_CORES          # 512 rows per core
RG = 128                         # batch rows per matmul group (= partitions)
NG = B_SH // RG                  # 4 groups per core
NKC = IN_F // 128                # 8 contraction chunks for the base matmul
NH = OUT_F // 512                # 2 output halves (PSUM bank free-dim limit)
NBG = B_SH // 128                # 4 PSUM row-blocks (== NG)

DR = True                        # fp8-e4m3 DoubleRow for the lora matmuls
DR_SUB = 2 if DR else 1          # k-subtiles per matmul (DoubleRow pairs)
C_SUB = R // DR_SUB              # r-chunks accumulated via separate matmuls
TAB_SCALE = 1.0 if DR else 64.0  # fp8 range centering (1/scale folded in ltab)

F32 = mybir.dt.float32
BF16 = mybir.dt.bfloat16
F8E4 = mybir.dt.float8e4         # e4m3
F8E3 = mybir.dt.float8e3         # e3m4
I32 = mybir.dt.int32
TAB_DT = F8E4 if DR else F8E3
BT_DT = F8E4 if DR else BF16
MACRO = R * OUT_F                # fp8 elements per gathered macro-row

_CACHE = {}


def _build(reps=1):
    nc = bacc.Bacc("TRN2", target_bir_lowering=False, debug=False,
                   num_devices=N_CORES)
    xt = nc.dram_tensor("xt", [IN_F, B_SH], BF16, kind="ExternalInput")
    wt = nc.dram_tensor("wt", [IN_F, OUT_F], BF16, kind="ExternalInput")
    a2w = nc.dram_tensor("a2w", [128, NKC * R], BF16, kind="ExternalInput")
    wct = nc.dram_tensor("wct", [R, OUT_F], BF16, kind="ExternalInput")
    biasb = nc.dram_tensor("biasb", [1, OUT_F], BF16, kind="ExternalInput")
    ones1 = nc.dram_tensor("ones1", [1, 128], BF16, kind="ExternalInput")
    ltab = nc.dram_tensor("ltab", [R, R * 128], BF16, kind="ExternalInput")
    masks = nc.dram_tensor("masks", [128, NG * RG], BF16, kind="ExternalInput")
    but = nc.dram_tensor("but", [NUM_USERS, MACRO], TAB_DT,
                         kind="ExternalInput")
    bit = nc.dram_tensor("bit", [NUM_ITEMS, MACRO], TAB_DT,
                         kind="ExternalInput")
    uidx = nc.dram_tensor("uidx", [128, NG], I32, kind="ExternalInput")
    iidx = nc.dram_tensor("iidx", [128, NG], I32, kind="ExternalInput")
    # y layout: [p, (bg, h, o)] -> row 128*bg+p, col 512*h+o of the [512,
    # 1024] shard. One contiguous 16 KiB row per partition => the per-body
    # output store is a single DMA with 16 KiB descriptors (128x 2 KiB
    # descriptors per bank would run at ~1/5 the engine rate).
    y = nc.dram_tensor("y", [128, NBG * OUT_F], F32, kind="ExternalOutput")

    perf_mode = mybir.MatmulPerfMode.DoubleRow if DR else None

    with tile.TileContext(nc) as tc:
        with (
            tc.tile_pool(name="const", bufs=1) as cp,
            tc.tile_pool(name="work", bufs=2) as wp,
            tc.tile_pool(name="gath", bufs=6) as gp,
            tc.tile_pool(name="ps", bufs=1, space="PSUM") as psp,
            tc.tile_pool(name="outp", bufs=2) as op,
        ):
            # ---- constant / weight loads (once) ----
            xt_t = []
            for k in range(NKC):
                t = cp.tile([128, B_SH], BF16, tag=f"xt{k}")
                nc.sync.dma_start(t[:], xt.ap()[128 * k:128 * (k + 1), :])
                xt_t.append(t)
            wt_t = []
            for k in range(NKC):
                t = cp.tile([128, OUT_F], BF16, tag=f"wt{k}")
                nc.sync.dma_start(t[:], wt.ap()[128 * k:128 * (k + 1), :])
                wt_t.append(t)
            a2w_t = cp.tile([128, NKC * R], BF16, tag="a2w")
            nc.sync.dma_start(a2w_t[:], a2w.ap())
            wct_t = cp.tile([R, OUT_F], BF16, tag="wct")
            nc.sync.dma_start(wct_t[:], wct.ap())
            bias_t = cp.tile([1, OUT_F], BF16, tag="bias")
            nc.sync.dma_start(bias_t[:], biasb.ap())
            ones_t = cp.tile([1, 128], BF16, tag="ones")
            nc.sync.dma_start(ones_t[:], ones1.ap())
            ltab_t = cp.tile([R, R * 128], BF16, tag="ltab")
            nc.sync.dma_start(ltab_t[:], ltab.ap())
            mask_t = cp.tile([128, NG, RG], BF16, tag="mask")
            nc.sync.dma_start(mask_t[:], masks.ap())
            uidx_t = cp.tile([128, NG], I32, tag="uidx")
            nc.sync.dma_start(uidx_t[:], uidx.ap())
            iidx_t = cp.tile([128, NG], I32, tag="iidx")
            nc.sync.dma_start(iidx_t[:], iidx.ap())

            def body():
                ot_all = op.tile([128, NBG, NH, 512], F32, tag="ot")

                # ---- a2T = (2A) @ x_shard.T  -> [16, 512] f32 -> bf16 ----
                a2t_ps = psp.tile([128, 512], F32, tag="a2t", bufs=1,
                                  space="PSUM")
                for k in range(NKC):
                    nc.tensor.matmul(
                        a2t_ps[:R, :], lhsT=a2w_t[:, R * k:R * (k + 1)],
                        rhs=xt_t[k][:],
                        start=(k == 0), stop=(k == NKC - 1),
                        skip_group_check=True)
                a2t_sb = wp.tile([R, B_SH], BF16, tag="a2t")
                nc.vector.tensor_copy(a2t_sb[:], a2t_ps[:R, :])

                # ---- bt: diagonal lhsT coefficients (batch row j <-> col j)
                # rep matmul broadcasts a2[ci] across partitions into PSUM;
                # the mask-mult reads it straight from PSUM.
                bt_all = []
                for c in range(C_SUB):
                    bt = wp.tile([128, NG, DR_SUB, RG], BT_DT, tag=f"bt{c}")
                    for i in range(DR_SUB):
                        ci = DR_SUB * c + i
                        rps = psp.tile([128, NG, RG], F32, tag="rep",
                                       bufs=2, space="PSUM")
                        nc.tensor.matmul(
                            rps[:].opt(), lhsT=ltab_t[:, 128 * ci:128 * (ci + 1)],
                            rhs=a2t_sb[:],
                            start=True, stop=True, skip_group_check=True)
                        nc.vector.tensor_tensor(
                            out=bt[:, :, i, :], in0=mask_t[:],
                            in1=rps[:],
                            op=mybir.AluOpType.mult)
                    bt_all.append(bt)

                # ---- per group: init PSUM bank (bias+base+common), gather
                # both table rows, run the diagonal lora matmuls, store. The
                # bank init is interleaved with the g loop so the PE reaches
                # the first lora matmul quickly and gather buffers recycle
                # without stalling the DMA engines.
                for g in range(NG):
                    ps_h = {}
                    for h in range(NH):
                        ps = psp.tile([128, 512], F32, tag="out",
                                      bufs=4, space="PSUM")
                        ps_h[h] = ps
                        nc.tensor.matmul(  # bias broadcast (K=1)
                            ps[:], lhsT=ones_t[:],
                            rhs=bias_t[:, 512 * h:512 * h + 512],
                            start=True, stop=False, skip_group_check=True)
                        for k in range(NKC):  # base: x @ W_base.T (bf16)
                            nc.tensor.matmul(
                                ps[:], lhsT=xt_t[k][:, 128 * g:128 * (g + 1)],
                                rhs=wt_t[k][:, 512 * h:512 * h + 512],
                                start=False, stop=False, skip_group_check=True)
                        nc.tensor.matmul(  # common: a2 @ W_common.T
                            ps[:], lhsT=a2t_sb[:, 128 * g:128 * (g + 1)],
                            rhs=wct_t[:, 512 * h:512 * h + 512],
                            start=False, stop=False, skip_group_check=True)
                    n_left = {h: 2 * C_SUB for h in range(NH)}
                    for tab_ap, idx_t in ((but.ap(), uidx_t), (bit.ap(), iidx_t)):
                        gt = gp.tile([128, C_SUB, DR_SUB, OUT_F], TAB_DT,
                                     tag="gt")
                        nc.gpsimd.indirect_dma_start(
                            out=gt[:].opt(), out_offset=None, in_=tab_ap,
                            in_offset=bass.IndirectOffsetOnAxis(
                                ap=idx_t[:, g:g + 1], axis=0))
                        for c in range(C_SUB):
                            for h in range(NH):
                                n_left[h] -= 1
                                nc.tensor.matmul(
                                    ps_h[h][:],
                                    lhsT=bt_all[c][:, g, :, :],
                                    rhs=gt[:, c, :, 512 * h:512 * h + 512],
                                    start=False, stop=(n_left[h] == 0),
                                    perf_mode=perf_mode,
                                    skip_group_check=True)
                    for h in range(NH):
                        nc.scalar.copy(
                            ot_all[:, g, h, :], ps_h[h][:])
                nc.sync.dma_start(y.ap(), ot_all[:].opt())

            for _ in range(reps):
                body()
    nc.compile()
    return nc


def _prep_host(x, user_indices, item_indices, W_base, b_base, A, B_user,
               B_item, W_common):
    """Host-side layout prep. Returns (shared dict, per-core list of dicts)."""
    bf16 = ml_dtypes.bfloat16
    tab_np = mybir.dt.np(TAB_DT)
    x = np.asarray(x, np.float32)
    W_base = np.asarray(W_base, np.float32)
    b_base = np.asarray(b_base, np.float32)
    A = np.asarray(A, np.float32)
    W_common = np.asarray(W_common, np.float32)
    user_indices = np.asarray(user_indices, np.int32)
    item_indices = np.asarray(item_indices, np.int32)

    wt = np.ascontiguousarray(W_base.T).astype(bf16)          # [in, out]
    a2t = np.ascontiguousarray((SCALING * A).T)               # [in, R]
    # a2w[p, R*k + r] = a2t[128k + p, r]
    a2w = np.ascontiguousarray(
        a2t.reshape(NKC, 128, R).transpose(1, 0, 2).reshape(128, NKC * R)
    ).astype(bf16)
    wct = np.ascontiguousarray(W_common.T).astype(bf16)       # [R, out]
    biasb = b_base.reshape(1, OUT_F).astype(bf16)
    ones1 = np.ones((1, 128), bf16)
    # ltab[r, 128*ci + p] = 1/TAB_SCALE iff r == ci  (broadcast a2[ci])
    ltab = np.zeros((R, R * 128), np.float32)
    for ci in range(R):
        ltab[ci, 128 * ci:128 * (ci + 1)] = 1.0 / TAB_SCALE
    ltab = ltab.astype(bf16)
    # masks[p, g, j] = 1 if p == j  (diagonal; same for every group)
    masks = np.zeros((128, NG, RG), np.float32)
    p = np.arange(128)
    masks[p, :, p] = 1.0
    masks = masks.reshape(128, NG * RG).astype(bf16)

    def mk_table(B, n):
        # [U, O, R] -> [U, R, O] with r-order (c, i): r = DR_SUB*c + i
        Bq = (np.asarray(B, np.float32) * TAB_SCALE).astype(tab_np)
        return np.ascontiguousarray(
            Bq.transpose(0, 2, 1)).reshape(n, MACRO)

    but = mk_table(B_user, NUM_USERS)
    bit = mk_table(B_item, NUM_ITEMS)

    shared = dict(wt=wt, a2w=a2w, wct=wct, biasb=np.asarray(biasb),
                  ones1=np.asarray(ones1), ltab=ltab, masks=masks,
                  but=but, bit=bit)
    per_core = []
    for cc in range(N_CORES):
        sl = slice(B_SH * cc, B_SH * (cc + 1))
        xt_c = np.ascontiguousarray(x[sl].T).astype(bf16)     # [in, 512]
        per_core.append(dict(
            xt=xt_c,
            uidx=np.ascontiguousarray(
                user_indices[sl].reshape(NG, RG).T),
            iidx=np.ascontiguousarray(
                item_indices[sl].reshape(NG, RG).T)))
    return shared, per_core


def kernel(**inputs) -> np.ndarray:
    if "nc" not in _CACHE:
        _CACHE["nc"] = _build()
    nc = _CACHE["nc"]
    shared, per_core = _prep_host(**inputs)
    in_maps = [{**shared, **pc} for pc in per_core]
    res = run_bass_kernel_spmd(nc, in_maps, core_ids=list(range(N_CORES)))
    shards = [_unshard_y(res.results[c]["y"]) for c in range(N_CORES)]
    return np.concatenate(shards, axis=0).astype(np.float32)


def _unshard_y(y_dev):
    return (y_dev.reshape(128, NBG, NH, 512)
            .transpose(1, 0, 2, 3).reshape(B_SH, OUT_F))
